# revision 70
# baseline (speedup 1.0000x reference)
"""Gumbel top-k (sequential masking) Trainium2 kernel, v6.

B=64 rows, N=16384, K=16 sequential top-1+mask steps; outputs st
(one-hot) and softs, each [K, B, N] f32 (softs emitted bf16, st u8).
Data-parallel: 8 rows/core x 8 cores; row = 16 partitions x 1024.
DRAM outputs partition-major; host transposes back.

v6: the DRAM staging/readback/gather pipeline of v4/v5 is gone.  The
row-broadcast shuffles carry each partition's candidate POSITION table
(chunk-local: half*512 + miu) alongside the values, so after
find_index produces flat candidate slots, every partition resolves its
scatter offsets locally: one scalar_tensor_tensor mask-gather
(iota==slot times table, fused accumulate) per consumer over tables
that already include the owning-partition iota ramp (slot>>4 *
16384|8192).  No staging DMA, no readbacks, no indirect gathers.

Structure:
  - selection runs on z (argmax invariant under exp), overlapping exp.
  - scales via two chained tensor_tensor_scans + reciprocals.
  - mr tree e0->e4->e8->e12 (exact e-values from a tiny ACT exp of the
    row-top z values); planes j%4!=0 copy from the group tile and get
    their stale top positions zeroed in DRAM by two q-packed indirect
    scatters (A: planes 1-3,5-7 -> slo; B: 9-11,13-15 -> shi).
  - planes: ACT g0+g2 (act copy w/ scale), DVE g1+g3 (tensor_scalar);
    each engine triggers its own span DMAs (sync triggers DVE's).
  - st = 2MB zero-fill DMA + one [P,1] scatter of ones.
"""

import numpy as np
from contextlib import ExitStack

import concourse.bacc as bacc
import concourse.bass as bass
import concourse.mybir as mybir
import concourse.tile as tile
from concourse.bass_utils import run_bass_kernel_spmd

F32 = mybir.dt.float32
BF16 = mybir.dt.bfloat16
U8 = mybir.dt.uint8
U32 = mybir.dt.uint32
AF = mybir.ActivationFunctionType
OP = mybir.AluOpType

B, N, NCORES = 64, 16384, 8
R = B // NCORES
QP = 16
FREE = N // QP           # 1024
P = 128
H = FREE // 2
INV_TAU = 1.5
K16 = 16
KF = K16 * FREE
NEG = -1.0e30
SW = 36                  # shuffled row width: 16 vals + 2 accums + 16 pos + 2 pad

# q-packed scatter items: (plane, rank) per q slot.  Pads duplicate the
# last real item (writing 0 twice is idempotent).
ITEMS_A = [(1, 0), (2, 0), (2, 1), (3, 0), (3, 1), (3, 2),
           (5, 4), (6, 4), (6, 5), (7, 4), (7, 5), (7, 6),
           (7, 6), (7, 6), (7, 6), (7, 6)]
ITEMS_B = [(9, 8), (10, 8), (10, 9), (11, 8), (11, 9), (11, 10),
           (13, 12), (14, 12), (14, 13), (15, 12), (15, 13), (15, 14),
           (15, 14), (15, 14), (15, 14), (15, 14)]

PLANE_ENG = {0: 'act', 1: 'act', 2: 'act', 3: 'act',
             4: 'dve', 5: 'dve', 6: 'dve', 7: 'dve',
             8: 'act', 9: 'act', 10: 'act', 11: 'act',
             12: 'dve', 13: 'dve', 14: 'dve', 15: 'dve'}

_module_cache = {}


def _host_consts():
    p = np.arange(P)
    q = p % 16
    r = p // 16
    cc = np.zeros((P, 8), np.uint32)
    cc[:, 0] = 0                         # position base, half 0 (chunk-local)
    cc[:, 1] = 512                       # position base, half 1
    cc[:, 2] = q                         # st rank pick
    ja = np.array([j for j, _ in ITEMS_A], np.uint32)
    ra = np.array([k for _, k in ITEMS_A], np.uint32)
    jb = np.array([j - 8 for j, _ in ITEMS_B], np.uint32)
    rbk = np.array([k for _, k in ITEMS_B], np.uint32)
    cc[:, 3] = ra[q]                     # A rank pick
    cc[:, 4] = rbk[q]                    # B rank pick
    cc[:, 5] = r * 16 * 16384 + q * 1024     # st flat base
    cc[:, 6] = r * 16 * 8192 + ja[q] * 1024  # A flat base (slo-local)
    cc[:, 7] = r * 16 * 8192 + jb[q] * 1024  # B flat base (shi-local)
    return cc


def _build16():
    nc = bacc.Bacc("TRN2", target_bir_lowering=False, debug=False,
                   num_devices=NCORES)
    zc_d = nc.dram_tensor("zc", [P, FREE + 8], F32, kind="ExternalInput")
    slo_d = nc.dram_tensor("slo", [P * KF // 2, 1], BF16,
                           kind="ExternalOutput")
    shi_d = nc.dram_tensor("shi", [P * KF // 2, 1], BF16,
                           kind="ExternalOutput")
    st_d = nc.dram_tensor("st", [P * KF, 1], U8, kind="ExternalOutput")

    slo_2d = slo_d.ap().rearrange("(p f) o -> p (f o)", p=P)
    shi_2d = shi_d.ap().rearrange("(p f) o -> p (f o)", p=P)
    st_2d = st_d.ap().rearrange("(p f) o -> p (f o)", p=P)

    with tile.TileContext(nc) as tc, ExitStack() as ctx:
        sp = ctx.enter_context(tc.tile_pool(name="sp", bufs=1))

        zlo = sp.tile([P, H], F32, tag="zlo")
        zhi = sp.tile([P, H + 8], F32, tag="zhi")
        cc = zhi[:, H:H + 8].bitcast(U32)
        e0 = sp.tile([P, FREE], F32, tag="e0")
        etiles = {0: e0}
        for t in (4, 8, 12):
            etiles[t] = sp.tile([P, FREE], F32, tag=f"e{t}", name=f"e{t}")
        softs_sb = sp.tile([P, KF], BF16, tag="softs_sb")
        stz = sp.tile([P, KF // 4], F32, tag="stz")
        selz = sp.tile([P, SW], F32, tag="selz")
        pos = selz[:, 18:34].bitcast(U32)
        miu = sp.tile([P, 16], U32, tag="miu")
        cand = sp.tile([P, 16 * SW], F32, tag="cand")
        vbr = sp.tile([P, 34], F32, tag="vbr")
        ec = sp.tile([P, 256], F32, tag="ec")
        c2 = sp.tile([P, 256], F32, tag="c2")
        postab = sp.tile([P, 256], U32, tag="postab")
        ptst = sp.tile([P, 256], U32, tag="ptst")
        ptab = sp.tile([P, 256], U32, tag="ptab")
        qt16k = sp.tile([P, 256], U32, tag="qt16k")
        qt8k = sp.tile([P, 256], U32, tag="qt8k")
        it256 = sp.tile([P, 256], U32, tag="it256")
        tf256 = sp.tile([P, 256], F32, tag="tf256")
        tf16 = sp.tile([P, 16], F32, tag="tf16")
        spick_f = sp.tile([P, 3], F32, tag="spick_f")
        opos_f = sp.tile([P, 3], F32, tag="opos_f")
        ekeys = sp.tile([P, 16], F32, tag="ekeys")
        padk = sp.tile([P, 24], F32, tag="padk")
        negt = sp.tile([P, 16], F32, tag="negt")
        SSp = sp.tile([P, 16], F32, tag="SSp")
        slots = sp.tile([P, 16], U32, tag="slots")
        spick = sp.tile([P, 3], U32, tag="spick")
        opos = sp.tile([P, 3], U32, tag="opos")
        otmp = sp.tile([P, 3], U32, tag="otmp")
        offd = sp.tile([P, 3], U32, tag="offd")
        ones = sp.tile([P, 1], U8, tag="ones")
        zbf = sp.tile([P, 1], BF16, tag="zbf")

        # ---- phase 0 -----------------------------------------------
        nc.gpsimd.memset(stz[:], 0.0)
        nc.vector.memset(padk[:], -1.0)
        nc.vector.memset(selz[:, 34:36], 0.0)
        nc.gpsimd.memset(ones[:], 1)
        nc.gpsimd.memset(zbf[:], 0.0)
        nc.gpsimd.iota(it256[:], [[1, 256]], base=0, channel_multiplier=0)
        # slot -> owning-partition offset ramps: (slot >> 4) * (16384|8192)
        nc.gpsimd.iota(qt16k[:].rearrange("p (a b) -> p a b", a=16),
                       [[16384, 16], [0, 16]], base=0, channel_multiplier=0)
        nc.gpsimd.iota(qt8k[:].rearrange("p (a b) -> p a b", a=16),
                       [[8192, 16], [0, 16]], base=0, channel_multiplier=0)

        nc.sync.dma_start(out=zlo[:], in_=zc_d.ap()[:, 0:H])
        nc.sync.dma_start(out=zhi[:], in_=zc_d.ap()[:, H:FREE + 8])
        nc.gpsimd.dma_start(out=st_2d, in_=stz[:].bitcast(U8))

        # ---- phase 1: per-partition selection on z + exp -----------
        nc.vector.max(selz[:, 0:8], zlo[:])
        nc.vector.max_index(miu[:, 0:8], selz[:, 0:8], zlo[:])
        nc.vector.max(selz[:, 8:16], zhi[:, 0:H])
        nc.vector.max_index(miu[:, 8:16], selz[:, 8:16], zhi[:, 0:H])
        nc.scalar.activation(e0[:, 0:H], zlo[:], AF.Exp, scale=INV_TAU,
                             accum_out=selz[:, 16:17])
        nc.scalar.activation(e0[:, H:FREE], zhi[:, 0:H], AF.Exp,
                             scale=INV_TAU, accum_out=selz[:, 17:18])
        # candidate positions within the row (ride along in the shuffle)
        nc.vector.tensor_tensor(pos[:, 0:8], miu[:, 0:8],
                                cc[:, 0:1].to_broadcast([P, 8]), OP.add)
        nc.vector.tensor_tensor(pos[:, 8:16], miu[:, 8:16],
                                cc[:, 1:2].to_broadcast([P, 8]), OP.add)

        # ---- phase 2: row-level selection (DVE) --------------------
        for q in range(QP):
            nc.vector.stream_shuffle(cand[:, SW * q:SW * q + SW], selz[:],
                                     [q] * 16 + [16 + q] * 16)
        gv = cand[:].rearrange("p (q c) -> p q c", c=SW)
        nc.vector.tensor_reduce(vbr[:, 32:33], gv[:, :, 16:18],
                                axis=mybir.AxisListType.XY, op=OP.add)
        nc.vector.reciprocal(vbr[:, 16:17], vbr[:, 32:33])   # 1/S0
        nc.vector.max(vbr[:, 0:8], gv[:, :, 0:16])           # z-top 0-7
        nc.vector.tensor_copy(ec[:].rearrange("p (q j) -> p q j", j=16),
                              gv[:, :, 0:16])
        nc.scalar.activation(ekeys[:, 0:8], vbr[:, 0:8], AF.Exp,
                             scale=INV_TAU)
        nc.vector.match_replace(c2[:], vbr[:, 0:8], ec[:], NEG)
        nc.vector.max(vbr[:, 8:16], c2[:])                   # z-top 8-15
        nc.scalar.activation(ekeys[:, 8:16], vbr[:, 8:16], AF.Exp,
                             scale=INV_TAU)

        # scales: SSp[:, j] = S_{j+1} = S0 - sum_{r<=j} etop_r
        nc.vector.tensor_scalar(negt[:, 0:8], ekeys[:, 0:8], -1.0, None,
                                OP.mult)
        nc.vector.tensor_tensor_scan(SSp[:, 0:8], negt[:, 0:8],
                                     negt[:, 0:8], vbr[:, 32:33],
                                     OP.add, OP.bypass)
        nc.vector.reciprocal(vbr[:, 17:25], SSp[:, 0:8])     # 1/S_1..8
        nc.vector.tensor_scalar(negt[:, 8:16], ekeys[:, 8:16], -1.0, None,
                                OP.mult)
        nc.vector.tensor_tensor_scan(SSp[:, 8:16], negt[:, 8:16],
                                     negt[:, 8:16], SSp[:, 7:8],
                                     OP.add, OP.bypass)
        nc.vector.reciprocal(vbr[:, 25:32], SSp[:, 8:15])    # 1/S_9..15

        # mr tree keys (padk prefilled -1); pad rewrites read reciprocal
        # outputs so the scale recips schedule before the mr tree.
        nc.vector.tensor_copy(padk[:, 0:4], ekeys[:, 0:4])
        nc.vector.tensor_scalar(padk[:, 4:5], vbr[:, 17:18], 0.0, -1.0,
                                OP.mult, OP.add)
        nc.vector.tensor_copy(padk[:, 8:12], ekeys[:, 4:8])
        nc.vector.tensor_copy(padk[:, 16:20], ekeys[:, 8:12])
        nc.vector.tensor_scalar(padk[:, 20:21], vbr[:, 25:26], 0.0, -1.0,
                                OP.mult, OP.add)

        # ---- mr tree (DVE) + planes + span DMAs --------------------
        nc.vector.match_replace(etiles[4][:], padk[:, 0:8], e0[:], 0.0)
        nc.vector.match_replace(etiles[8][:], padk[:, 8:16], etiles[4][:],
                                0.0)
        nc.vector.match_replace(etiles[12][:], padk[:, 16:24],
                                etiles[8][:], 0.0)

        def plane(j):
            src = etiles[4 * (j // 4)]
            dst = softs_sb[:, j * FREE:(j + 1) * FREE]
            scl = vbr[:, 16 + j:17 + j]
            if PLANE_ENG[j] == 'act':
                nc.scalar.activation(dst, src[:], AF.Copy, scale=scl)
            else:
                nc.vector.tensor_scalar(dst, src[:], scl, None, OP.mult)

        def span_dma(eng, j0, n):
            tgt = slo_2d if j0 < 8 else shi_2d
            toff = (j0 % 8) * FREE
            eng.dma_start(
                out=tgt[:, toff:toff + n * FREE],
                in_=softs_sb[:, j0 * FREE:(j0 + n) * FREE])

        # fixup planes first per engine; exact plane (j%4==0) last.
        # Group 3 splits into (13,14,15) + (12) so scatter B never
        # waits on plane 12's DMA.
        for j in (1, 2, 3, 0):           # ACT g0
            plane(j)
        span_dma(nc.scalar, 0, 4)
        for j in (5, 6, 7, 4):           # DVE g1
            plane(j)
        span_dma(nc.sync, 4, 4)
        for j in (9, 10, 11, 8):         # ACT g2
            plane(j)
        span_dma(nc.scalar, 8, 4)
        for j in (13, 14, 15):           # DVE g3 fixups
            plane(j)
        span_dma(nc.sync, 13, 3)

        # ---- scatter offsets (emitted after the planes so the DVE
        # list scheduler keeps the scale chain + mr tree dense; the
        # scatters only need these at ~t+30us) ------------------------
        nc.vector.max_index(slots[:, 0:8], vbr[:, 0:8], ec[:])
        nc.vector.max_index(slots[:, 8:16], vbr[:, 8:16], c2[:])
        nc.vector.tensor_copy(
            postab[:].rearrange("p (q j) -> p q j", j=16),
            gv[:, :, 18:34].bitcast(U32))
        # gather tables with the owning-partition term baked in: the
        # accumulated value is the complete flat-local offset.
        nc.vector.tensor_tensor(ptst[:], postab[:], qt16k[:], OP.add)
        nc.vector.tensor_tensor(ptab[:], postab[:], qt8k[:], OP.add)
        for x in range(3):
            nc.vector.scalar_tensor_tensor(
                tf16[:], it256[:, 0:16], cc[:, 2 + x:3 + x], slots[:],
                OP.is_equal, OP.mult, accum_out=spick_f[:, x:x + 1])
        nc.vector.tensor_copy(spick[:], spick_f[:])
        for x, tab in ((0, ptst), (1, ptab), (2, ptab)):
            nc.vector.scalar_tensor_tensor(
                tf256[:], it256[:], spick[:, x:x + 1], tab[:],
                OP.is_equal, OP.mult, accum_out=opos_f[:, x:x + 1])
        nc.vector.tensor_copy(opos[:], opos_f[:])
        nc.gpsimd.tensor_tensor(otmp[:, 0:3], opos[:], cc[:, 5:8],
                                OP.add)

        # ---- indirect scatters (gps), in token-readiness order -----
        # st: waits zero-fill DMA
        nc.gpsimd.memset(stz[:, 0:1], 0.0)
        nc.gpsimd.tensor_tensor(offd[:, 0:1], otmp[:, 0:1],
                                stz[:, 0:1].bitcast(U32), OP.add)
        nc.gpsimd.indirect_dma_start(
            out=st_d.ap(),
            out_offset=bass.IndirectOffsetOnAxis(ap=offd[:, 0:1], axis=0),
            in_=ones[:], in_offset=None)
        # A: waits g0 + g1 DMAs
        nc.gpsimd.memset(softs_sb[:, 0:2], 0.0)
        nc.gpsimd.memset(softs_sb[:, 4 * FREE:4 * FREE + 2], 0.0)
        nc.gpsimd.tensor_tensor(offd[:, 1:2], otmp[:, 1:2],
                                softs_sb[:, 0:2].bitcast(U32), OP.add)
        nc.gpsimd.tensor_tensor(
            offd[:, 1:2], offd[:, 1:2],
            softs_sb[:, 4 * FREE:4 * FREE + 2].bitcast(U32), OP.add)
        nc.gpsimd.indirect_dma_start(
            out=slo_d.ap(),
            out_offset=bass.IndirectOffsetOnAxis(ap=offd[:, 1:2], axis=0),
            in_=zbf[:], in_offset=None)
        # B: waits g2 + g3a DMAs (not plane 12's)
        nc.gpsimd.memset(softs_sb[:, 8 * FREE:8 * FREE + 2], 0.0)
        nc.gpsimd.memset(softs_sb[:, 13 * FREE:13 * FREE + 2], 0.0)
        nc.gpsimd.tensor_tensor(offd[:, 2:3], otmp[:, 2:3],
                                softs_sb[:, 8 * FREE:8 * FREE + 2]
                                .bitcast(U32), OP.add)
        nc.gpsimd.tensor_tensor(
            offd[:, 2:3], offd[:, 2:3],
            softs_sb[:, 13 * FREE:13 * FREE + 2].bitcast(U32), OP.add)
        nc.gpsimd.indirect_dma_start(
            out=shi_d.ap(),
            out_offset=bass.IndirectOffsetOnAxis(ap=offd[:, 2:3], axis=0),
            in_=zbf[:], in_offset=None)

        # plane 12 (exact, nothing waits on it) emitted dead last so
        # scatter B's conservative sync-queue completion threshold
        # stops at dma(13-15) instead of including dma(12)
        plane(12)
        span_dma(nc.sync, 12, 1)
    nc.compile()
    return nc


def kernel(logits, gumbel, k, trace=False):
    K = int(k)
    logits = np.ascontiguousarray(logits, dtype=np.float32)
    gumbel = np.ascontiguousarray(gumbel, dtype=np.float32)
    if K == 0:
        empty = np.zeros((0, B, N), dtype=np.float32)
        return empty, empty.copy()
    assert K == 16, f"kernel supports k=16 only, got {K}"
    assert logits.shape == (B, N) and gumbel.shape == (B, N)

    if K not in _module_cache:
        _module_cache[K] = _build16()
    nc = _module_cache[K]

    cc = _host_consts().view(np.float32)
    z_full = logits + gumbel
    in_maps = []
    for c in range(NCORES):
        sl = slice(c * R, (c + 1) * R)
        zc = np.concatenate([z_full[sl].reshape(P, FREE), cc], axis=1)
        in_maps.append({"zc": np.ascontiguousarray(zc)})

    res = run_bass_kernel_spmd(nc, in_maps, core_ids=list(range(NCORES)),
                               trace=trace)

    st = np.empty((K, B, N), dtype=np.float32)
    softs = np.empty((K, B, N), dtype=np.float32)
    for c in range(NCORES):
        sl = slice(c * R, (c + 1) * R)
        lo = res.results[c]["slo"].reshape(R, QP, 8, FREE)
        hi = res.results[c]["shi"].reshape(R, QP, 8, FREE)
        s = np.concatenate([lo, hi], axis=2)
        softs[:, sl, :] = np.transpose(s.astype(np.float32), (2, 0, 1, 3)) \
            .reshape(K, R, N)
        t = res.results[c]["st"].reshape(R, QP, K16, FREE)
        st[:, sl, :] = np.transpose(t, (2, 0, 1, 3)).reshape(K16, R, N) \
            .astype(np.float32)

    if trace:
        kernel.last_exec_time_ns = res.exec_time_ns
        kernel.last_results = res
    return st, softs


# revision 71
# speedup vs baseline: 1.0025x; 1.0025x over previous
"""Gumbel top-k (sequential masking) Trainium2 kernel, v6.

B=64 rows, N=16384, K=16 sequential top-1+mask steps; outputs st
(one-hot) and softs, each [K, B, N] f32 (softs emitted bf16, st u8).
Data-parallel: 8 rows/core x 8 cores; row = 16 partitions x 1024.
DRAM outputs partition-major; host transposes back.

v6: the DRAM staging/readback/gather pipeline of v4/v5 is gone.  The
row-broadcast shuffles carry each partition's candidate POSITION table
(chunk-local: half*512 + miu) alongside the values, so after
find_index produces flat candidate slots, every partition resolves its
scatter offsets locally: one scalar_tensor_tensor mask-gather
(iota==slot times table, fused accumulate) per consumer over tables
that already include the owning-partition iota ramp (slot>>4 *
16384|8192).  No staging DMA, no readbacks, no indirect gathers.

Structure:
  - selection runs on z (argmax invariant under exp), overlapping exp.
  - scales via two chained tensor_tensor_scans + reciprocals.
  - mr tree e0->e4->e8->e12 (exact e-values from a tiny ACT exp of the
    row-top z values); planes j%4!=0 copy from the group tile and get
    their stale top positions zeroed in DRAM by two q-packed indirect
    scatters (A: planes 1-3,5-7 -> slo; B: 9-11,13-15 -> shi).
  - planes: ACT g0+g2 (act copy w/ scale), DVE g1+g3 (tensor_scalar);
    each engine triggers its own span DMAs (sync triggers DVE's).
  - st = 2MB zero-fill DMA + one [P,1] scatter of ones.
"""

import numpy as np
from contextlib import ExitStack

import concourse.bacc as bacc
import concourse.bass as bass
import concourse.mybir as mybir
import concourse.tile as tile
from concourse.bass_utils import run_bass_kernel_spmd

F32 = mybir.dt.float32
BF16 = mybir.dt.bfloat16
U8 = mybir.dt.uint8
U32 = mybir.dt.uint32
AF = mybir.ActivationFunctionType
OP = mybir.AluOpType

B, N, NCORES = 64, 16384, 8
R = B // NCORES
QP = 16
FREE = N // QP           # 1024
P = 128
H = FREE // 2
INV_TAU = 1.5
K16 = 16
KF = K16 * FREE
NEG = -1.0e30
SW = 36                  # shuffled row width: 16 vals + 2 accums + 16 pos + 2 pad

# q-packed scatter items: (plane, rank) per q slot.  Pads duplicate the
# last real item (writing 0 twice is idempotent).
ITEMS_A = [(1, 0), (2, 0), (2, 1), (3, 0), (3, 1), (3, 2),
           (5, 4), (6, 4), (6, 5), (7, 4), (7, 5), (7, 6),
           (7, 6), (7, 6), (7, 6), (7, 6)]
ITEMS_B = [(9, 8), (10, 8), (10, 9), (11, 8), (11, 9), (11, 10),
           (13, 12), (14, 12), (14, 13), (15, 12), (15, 13), (15, 14),
           (15, 14), (15, 14), (15, 14), (15, 14)]

PLANE_ENG = {0: 'act', 1: 'act', 2: 'act', 3: 'act',
             4: 'dve', 5: 'dve', 6: 'dve', 7: 'dve',
             8: 'act', 9: 'act', 10: 'act', 11: 'act',
             12: 'dve', 13: 'dve', 14: 'dve', 15: 'dve'}

_module_cache = {}


def _host_consts():
    p = np.arange(P)
    q = p % 16
    r = p // 16
    cc = np.zeros((P, 8), np.uint32)
    cc[:, 0] = 0                         # position base, half 0 (chunk-local)
    cc[:, 1] = 512                       # position base, half 1
    cc[:, 2] = q                         # st rank pick
    ja = np.array([j for j, _ in ITEMS_A], np.uint32)
    ra = np.array([k for _, k in ITEMS_A], np.uint32)
    jb = np.array([j - 8 for j, _ in ITEMS_B], np.uint32)
    rbk = np.array([k for _, k in ITEMS_B], np.uint32)
    cc[:, 3] = ra[q]                     # A rank pick
    cc[:, 4] = rbk[q]                    # B rank pick
    cc[:, 5] = r * 16 * 16384 + q * 1024     # st flat base
    cc[:, 6] = r * 16 * 8192 + ja[q] * 1024  # A flat base (slo-local)
    cc[:, 7] = r * 16 * 8192 + jb[q] * 1024  # B flat base (shi-local)
    return cc


def _build16():
    nc = bacc.Bacc("TRN2", target_bir_lowering=False, debug=False,
                   num_devices=NCORES)
    zc_d = nc.dram_tensor("zc", [P, FREE + 8], F32, kind="ExternalInput")
    slo_d = nc.dram_tensor("slo", [P * KF // 2, 1], BF16,
                           kind="ExternalOutput")
    shi_d = nc.dram_tensor("shi", [P * KF // 2, 1], BF16,
                           kind="ExternalOutput")
    st_d = nc.dram_tensor("st", [P * KF, 1], U8, kind="ExternalOutput")

    slo_2d = slo_d.ap().rearrange("(p f) o -> p (f o)", p=P)
    shi_2d = shi_d.ap().rearrange("(p f) o -> p (f o)", p=P)
    st_2d = st_d.ap().rearrange("(p f) o -> p (f o)", p=P)

    with tile.TileContext(nc) as tc, ExitStack() as ctx:
        sp = ctx.enter_context(tc.tile_pool(name="sp", bufs=1))

        zlo = sp.tile([P, H], F32, tag="zlo")
        zhi = sp.tile([P, H + 8], F32, tag="zhi")
        cc = zhi[:, H:H + 8].bitcast(U32)
        e0 = sp.tile([P, FREE], F32, tag="e0")
        etiles = {0: e0}
        for t in (4, 8, 12):
            etiles[t] = sp.tile([P, FREE], F32, tag=f"e{t}", name=f"e{t}")
        softs_sb = sp.tile([P, KF], BF16, tag="softs_sb")
        stz = sp.tile([P, KF // 4], F32, tag="stz")
        selz = sp.tile([P, SW], F32, tag="selz")
        pos = selz[:, 18:34].bitcast(U32)
        miu = sp.tile([P, 16], U32, tag="miu")
        cand = sp.tile([P, 16 * SW], F32, tag="cand")
        vbr = sp.tile([P, 34], F32, tag="vbr")
        ec = sp.tile([P, 256], F32, tag="ec")
        c2 = sp.tile([P, 256], F32, tag="c2")
        postab = sp.tile([P, 256], U32, tag="postab")
        ptst = sp.tile([P, 256], U32, tag="ptst")
        ptab = sp.tile([P, 256], U32, tag="ptab")
        qt16k = sp.tile([P, 256], U32, tag="qt16k")
        qt8k = sp.tile([P, 256], U32, tag="qt8k")
        it256 = sp.tile([P, 256], U32, tag="it256")
        tf256 = sp.tile([P, 256], F32, tag="tf256")
        tf16 = sp.tile([P, 16], F32, tag="tf16")
        spick_f = sp.tile([P, 3], F32, tag="spick_f")
        opos_f = sp.tile([P, 3], F32, tag="opos_f")
        ekeys = sp.tile([P, 16], F32, tag="ekeys")
        padk = sp.tile([P, 24], F32, tag="padk")
        negt = sp.tile([P, 16], F32, tag="negt")
        SSp = sp.tile([P, 16], F32, tag="SSp")
        slots = sp.tile([P, 16], U32, tag="slots")
        spick = sp.tile([P, 3], U32, tag="spick")
        opos = sp.tile([P, 3], U32, tag="opos")
        otmp = sp.tile([P, 3], U32, tag="otmp")
        offd = sp.tile([P, 3], U32, tag="offd")
        ones = sp.tile([P, 1], U8, tag="ones")
        zbf = sp.tile([P, 1], BF16, tag="zbf")

        # ---- phase 0 -----------------------------------------------
        nc.gpsimd.memset(stz[:], 0.0)
        nc.vector.memset(padk[:], -1.0)
        nc.vector.memset(selz[:, 34:36], 0.0)
        nc.gpsimd.memset(ones[:], 1)
        nc.gpsimd.memset(zbf[:], 0.0)
        nc.gpsimd.iota(it256[:], [[1, 256]], base=0, channel_multiplier=0)
        # slot -> owning-partition offset ramps: (slot >> 4) * (16384|8192)
        nc.gpsimd.iota(qt16k[:].rearrange("p (a b) -> p a b", a=16),
                       [[16384, 16], [0, 16]], base=0, channel_multiplier=0)
        nc.gpsimd.iota(qt8k[:].rearrange("p (a b) -> p a b", a=16),
                       [[8192, 16], [0, 16]], base=0, channel_multiplier=0)

        nc.sync.dma_start(out=zlo[:], in_=zc_d.ap()[:, 0:H])
        nc.sync.dma_start(out=zhi[:], in_=zc_d.ap()[:, H:FREE + 8])
        nc.gpsimd.dma_start(out=st_2d, in_=stz[:].bitcast(U8))

        # ---- phase 1: per-partition selection on z + exp -----------
        nc.vector.max(selz[:, 0:8], zlo[:])
        nc.vector.max_index(miu[:, 0:8], selz[:, 0:8], zlo[:])
        nc.vector.max(selz[:, 8:16], zhi[:, 0:H])
        nc.vector.max_index(miu[:, 8:16], selz[:, 8:16], zhi[:, 0:H])
        nc.scalar.activation(e0[:, 0:H], zlo[:], AF.Exp, scale=INV_TAU,
                             accum_out=selz[:, 16:17])
        nc.scalar.activation(e0[:, H:FREE], zhi[:, 0:H], AF.Exp,
                             scale=INV_TAU, accum_out=selz[:, 17:18])
        # candidate positions within the row (ride along in the shuffle)
        nc.vector.tensor_tensor(pos[:, 0:8], miu[:, 0:8],
                                cc[:, 0:1].to_broadcast([P, 8]), OP.add)
        nc.vector.tensor_tensor(pos[:, 8:16], miu[:, 8:16],
                                cc[:, 1:2].to_broadcast([P, 8]), OP.add)

        # ---- phase 2: row-level selection (DVE) --------------------
        for q in range(QP):
            nc.vector.stream_shuffle(cand[:, SW * q:SW * q + SW], selz[:],
                                     [q] * 16 + [16 + q] * 16)
        gv = cand[:].rearrange("p (q c) -> p q c", c=SW)
        nc.vector.tensor_reduce(vbr[:, 32:33], gv[:, :, 16:18],
                                axis=mybir.AxisListType.XY, op=OP.add)
        nc.vector.reciprocal(vbr[:, 16:17], vbr[:, 32:33])   # 1/S0
        nc.vector.max(vbr[:, 0:8], gv[:, :, 0:16])           # z-top 0-7
        nc.vector.tensor_copy(ec[:].rearrange("p (q j) -> p q j", j=16),
                              gv[:, :, 0:16])
        nc.scalar.activation(ekeys[:, 0:8], vbr[:, 0:8], AF.Exp,
                             scale=INV_TAU)
        nc.vector.match_replace(c2[:], vbr[:, 0:8], ec[:], NEG)
        nc.vector.max(vbr[:, 8:16], c2[:])                   # z-top 8-15
        nc.scalar.activation(ekeys[:, 8:16], vbr[:, 8:16], AF.Exp,
                             scale=INV_TAU)

        # scales: SSp[:, j] = S_{j+1} = S0 - sum_{r<=j} etop_r
        nc.vector.tensor_scalar(negt[:, 0:8], ekeys[:, 0:8], -1.0, None,
                                OP.mult)
        nc.vector.tensor_tensor_scan(SSp[:, 0:8], negt[:, 0:8],
                                     negt[:, 0:8], vbr[:, 32:33],
                                     OP.add, OP.bypass)
        nc.vector.reciprocal(vbr[:, 17:25], SSp[:, 0:8])     # 1/S_1..8
        nc.vector.tensor_scalar(negt[:, 8:16], ekeys[:, 8:16], -1.0, None,
                                OP.mult)
        nc.vector.tensor_tensor_scan(SSp[:, 8:16], negt[:, 8:16],
                                     negt[:, 8:16], SSp[:, 7:8],
                                     OP.add, OP.bypass)
        nc.vector.reciprocal(vbr[:, 25:32], SSp[:, 8:15])    # 1/S_9..15

        # mr tree keys (padk prefilled -1); pad rewrites read reciprocal
        # outputs so the scale recips schedule before the mr tree.
        nc.vector.tensor_copy(padk[:, 0:4], ekeys[:, 0:4])
        nc.vector.tensor_scalar(padk[:, 4:5], vbr[:, 17:18], 0.0, -1.0,
                                OP.mult, OP.add)
        nc.vector.tensor_copy(padk[:, 8:12], ekeys[:, 4:8])
        nc.vector.tensor_copy(padk[:, 16:20], ekeys[:, 8:12])
        nc.vector.tensor_scalar(padk[:, 20:21], vbr[:, 25:26], 0.0, -1.0,
                                OP.mult, OP.add)

        # ---- mr tree (DVE) + planes + span DMAs --------------------
        nc.vector.match_replace(etiles[4][:], padk[:, 0:8], e0[:], 0.0)
        nc.vector.match_replace(etiles[8][:], padk[:, 8:16], etiles[4][:],
                                0.0)
        nc.vector.match_replace(etiles[12][:], padk[:, 16:24],
                                etiles[8][:], 0.0)

        def plane(j):
            src = etiles[4 * (j // 4)]
            dst = softs_sb[:, j * FREE:(j + 1) * FREE]
            scl = vbr[:, 16 + j:17 + j]
            if PLANE_ENG[j] == 'act':
                nc.scalar.activation(dst, src[:], AF.Copy, scale=scl)
            else:
                nc.vector.tensor_scalar(dst, src[:], scl, None, OP.mult)

        def span_dma(eng, j0, n):
            tgt = slo_2d if j0 < 8 else shi_2d
            toff = (j0 % 8) * FREE
            eng.dma_start(
                out=tgt[:, toff:toff + n * FREE],
                in_=softs_sb[:, j0 * FREE:(j0 + n) * FREE])

        # fixup planes first per engine; exact plane (j%4==0) last.
        # Group 3 splits into (13,14,15) + (12) so scatter B never
        # waits on plane 12's DMA.
        for j in (1, 2, 3, 0):           # ACT g0
            plane(j)
        span_dma(nc.scalar, 0, 4)
        for j in (5, 6, 7, 4):           # DVE g1
            plane(j)
        span_dma(nc.sync, 4, 4)
        for j in (9, 10, 11, 8):         # ACT g2
            plane(j)
        span_dma(nc.scalar, 8, 4)
        for j in (13, 14, 15):           # DVE g3 fixups
            plane(j)
        span_dma(nc.sync, 13, 3)
        plane(12)
        span_dma(nc.sync, 12, 1)

        # ---- scatter offsets (emitted after the planes so the DVE
        # list scheduler keeps the scale chain + mr tree dense; the
        # scatters only need these at ~t+30us) ------------------------
        nc.vector.max_index(slots[:, 0:8], vbr[:, 0:8], ec[:])
        nc.vector.max_index(slots[:, 8:16], vbr[:, 8:16], c2[:])
        nc.vector.tensor_copy(
            postab[:].rearrange("p (q j) -> p q j", j=16),
            gv[:, :, 18:34].bitcast(U32))
        # gather tables with the owning-partition term baked in: the
        # accumulated value is the complete flat-local offset.
        nc.vector.tensor_tensor(ptst[:], postab[:], qt16k[:], OP.add)
        nc.vector.tensor_tensor(ptab[:], postab[:], qt8k[:], OP.add)
        for x in range(3):
            nc.vector.scalar_tensor_tensor(
                tf16[:], it256[:, 0:16], cc[:, 2 + x:3 + x], slots[:],
                OP.is_equal, OP.mult, accum_out=spick_f[:, x:x + 1])
        nc.vector.tensor_copy(spick[:], spick_f[:])
        for x, tab in ((0, ptst), (1, ptab), (2, ptab)):
            nc.vector.scalar_tensor_tensor(
                tf256[:], it256[:], spick[:, x:x + 1], tab[:],
                OP.is_equal, OP.mult, accum_out=opos_f[:, x:x + 1])
        nc.vector.tensor_copy(opos[:], opos_f[:])
        nc.gpsimd.tensor_tensor(otmp[:, 0:3], opos[:], cc[:, 5:8],
                                OP.add)

        # ---- indirect scatters (gps), in token-readiness order -----
        # st: waits zero-fill DMA
        nc.gpsimd.memset(stz[:, 0:1], 0.0)
        nc.gpsimd.tensor_tensor(offd[:, 0:1], otmp[:, 0:1],
                                stz[:, 0:1].bitcast(U32), OP.add)
        nc.gpsimd.indirect_dma_start(
            out=st_d.ap(),
            out_offset=bass.IndirectOffsetOnAxis(ap=offd[:, 0:1], axis=0),
            in_=ones[:], in_offset=None)
        # A: waits g0 + g1 DMAs
        nc.gpsimd.memset(softs_sb[:, 0:2], 0.0)
        nc.gpsimd.memset(softs_sb[:, 4 * FREE:4 * FREE + 2], 0.0)
        nc.gpsimd.tensor_tensor(offd[:, 1:2], otmp[:, 1:2],
                                softs_sb[:, 0:2].bitcast(U32), OP.add)
        nc.gpsimd.tensor_tensor(
            offd[:, 1:2], offd[:, 1:2],
            softs_sb[:, 4 * FREE:4 * FREE + 2].bitcast(U32), OP.add)
        nc.gpsimd.indirect_dma_start(
            out=slo_d.ap(),
            out_offset=bass.IndirectOffsetOnAxis(ap=offd[:, 1:2], axis=0),
            in_=zbf[:], in_offset=None)
        # B: waits g2 + g3a DMAs (not plane 12's)
        nc.gpsimd.memset(softs_sb[:, 8 * FREE:8 * FREE + 2], 0.0)
        nc.gpsimd.memset(softs_sb[:, 13 * FREE:13 * FREE + 2], 0.0)
        nc.gpsimd.tensor_tensor(offd[:, 2:3], otmp[:, 2:3],
                                softs_sb[:, 8 * FREE:8 * FREE + 2]
                                .bitcast(U32), OP.add)
        nc.gpsimd.tensor_tensor(
            offd[:, 2:3], offd[:, 2:3],
            softs_sb[:, 13 * FREE:13 * FREE + 2].bitcast(U32), OP.add)
        nc.gpsimd.indirect_dma_start(
            out=shi_d.ap(),
            out_offset=bass.IndirectOffsetOnAxis(ap=offd[:, 2:3], axis=0),
            in_=zbf[:], in_offset=None)
    nc.compile()
    return nc


def kernel(logits, gumbel, k, trace=False):
    K = int(k)
    logits = np.ascontiguousarray(logits, dtype=np.float32)
    gumbel = np.ascontiguousarray(gumbel, dtype=np.float32)
    if K == 0:
        empty = np.zeros((0, B, N), dtype=np.float32)
        return empty, empty.copy()
    assert K == 16, f"kernel supports k=16 only, got {K}"
    assert logits.shape == (B, N) and gumbel.shape == (B, N)

    if K not in _module_cache:
        _module_cache[K] = _build16()
    nc = _module_cache[K]

    cc = _host_consts().view(np.float32)
    z_full = logits + gumbel
    in_maps = []
    for c in range(NCORES):
        sl = slice(c * R, (c + 1) * R)
        zc = np.concatenate([z_full[sl].reshape(P, FREE), cc], axis=1)
        in_maps.append({"zc": np.ascontiguousarray(zc)})

    res = run_bass_kernel_spmd(nc, in_maps, core_ids=list(range(NCORES)),
                               trace=trace)

    st = np.empty((K, B, N), dtype=np.float32)
    softs = np.empty((K, B, N), dtype=np.float32)
    for c in range(NCORES):
        sl = slice(c * R, (c + 1) * R)
        lo = res.results[c]["slo"].reshape(R, QP, 8, FREE)
        hi = res.results[c]["shi"].reshape(R, QP, 8, FREE)
        s = np.concatenate([lo, hi], axis=2)
        softs[:, sl, :] = np.transpose(s.astype(np.float32), (2, 0, 1, 3)) \
            .reshape(K, R, N)
        t = res.results[c]["st"].reshape(R, QP, K16, FREE)
        st[:, sl, :] = np.transpose(t, (2, 0, 1, 3)).reshape(K16, R, N) \
            .astype(np.float32)

    if trace:
        kernel.last_exec_time_ns = res.exec_time_ns
        kernel.last_results = res
    return st, softs
